# revision 1
# baseline (speedup 1.0000x reference)
"""NeighbourChannels kernel for Trainium2 (8 NeuronCores, SPMD data-parallel).

out[b,c,h,w] = sum_j x[b,j,h,w] - x[b,c,h,w]   for x [16, 256, 128, 128] fp32.

Sharding: batch dim 16 -> 2 images per core across 8 cores (no cross-pixel or
cross-batch dependence).

Per-core Bass/Tile program, x viewed as [2 b][2 half][128 ch][16384 hw]:
  - stream [128, FREE] fp32 tiles for each channel-half (contiguous 4*FREE-byte
    runs per partition -> near-peak DMA efficiency)
  - channel-sum + partition-broadcast in one PE op per 512-pixel subchunk:
      psum[128,512] = onesT[128,128] @ A_sub + onesT @ B_sub   (PSUM accumulate)
    (every row of psum = per-pixel total over all 256 channels)
  - out = psum - x on VectorE
  - loads issued from SyncE (HWDGE), stores from ScalarE (second HWDGE ring)
"""

import numpy as np

B_TOTAL = 16
N_CORES = 8
B_PER_CORE = B_TOTAL // N_CORES
C = 256
HALF = 128
H = 128
W = 128
HW = H * W
FREE = 4096          # pixels per streamed tile (2 MiB per DMA)
SUB = 512            # pixels per PSUM bank / matmul moving tile
NSUB = FREE // SUB

_nc_cache = []


def _build_program():
    import concourse.bass as bass  # noqa: F401
    import concourse.tile as tile
    from concourse import bacc, mybir

    fp32 = mybir.dt.float32
    nc = bacc.Bacc(
        "TRN2",
        target_bir_lowering=False,
        debug=False,
        enable_asserts=False,
        num_devices=N_CORES,
    )
    x_ext = nc.dram_tensor(
        "x", [B_PER_CORE, 2, HALF, HW], fp32, kind="ExternalInput"
    )
    out_ext = nc.dram_tensor(
        "out", [B_PER_CORE, 2, HALF, HW], fp32, kind="ExternalOutput"
    )

    with tile.TileContext(nc) as tc:
        with (
            tc.tile_pool(name="const", bufs=1) as cpool,
            tc.tile_pool(name="io", bufs=2) as io_pool,
            tc.tile_pool(name="psum", bufs=8, space="PSUM") as psum_pool,
        ):
            ones = cpool.tile([128, 128], fp32, tag="ones")
            nc.vector.memset(ones[:], 1.0)
            for b in range(B_PER_CORE):
                for j in range(HW // FREE):
                    sl = slice(j * FREE, (j + 1) * FREE)
                    ta = io_pool.tile([128, FREE], fp32, tag="in_a")
                    nc.sync.dma_start(ta[:], x_ext[b, 0][:, sl])
                    tb = io_pool.tile([128, FREE], fp32, tag="in_b")
                    nc.sync.dma_start(tb[:], x_ext[b, 1][:, sl])
                    oa = io_pool.tile([128, FREE], fp32, tag="out_a")
                    ob = io_pool.tile([128, FREE], fp32, tag="out_b")
                    for s in range(NSUB):
                        ss = slice(s * SUB, (s + 1) * SUB)
                        ps = psum_pool.tile([128, SUB], fp32, tag="ps")
                        nc.tensor.matmul(
                            ps[:], ones[:], ta[:, ss], start=True, stop=False
                        )
                        nc.tensor.matmul(
                            ps[:], ones[:], tb[:, ss], start=False, stop=True
                        )
                        nc.vector.tensor_sub(oa[:, ss], ps[:], ta[:, ss])
                        nc.vector.tensor_sub(ob[:, ss], ps[:], tb[:, ss])
                    nc.scalar.dma_start(out_ext[b, 0][:, sl], oa[:])
                    nc.scalar.dma_start(out_ext[b, 1][:, sl], ob[:])
    nc.compile()
    return nc


def _get_program():
    if not _nc_cache:
        _nc_cache.append(_build_program())
    return _nc_cache[0]


def shard_inputs(x: np.ndarray) -> list[dict]:
    x = np.ascontiguousarray(np.asarray(x, dtype=np.float32))
    assert x.shape == (B_TOTAL, C, H, W), x.shape
    return [
        {
            "x": np.ascontiguousarray(
                x[i * B_PER_CORE : (i + 1) * B_PER_CORE]
            ).reshape(B_PER_CORE, 2, HALF, HW)
        }
        for i in range(N_CORES)
    ]


def unshard_outputs(results: list[dict]) -> np.ndarray:
    outs = [
        np.asarray(r["out"], dtype=np.float32).reshape(B_PER_CORE, C, H, W)
        for r in results
    ]
    return np.concatenate(outs, axis=0)


def kernel(x: np.ndarray) -> np.ndarray:
    from concourse.bass_utils import run_bass_kernel_spmd

    nc = _get_program()
    in_maps = shard_inputs(x)
    res = run_bass_kernel_spmd(nc, in_maps, list(range(N_CORES)))
    return unshard_outputs(res.results)
